# revision 2
# baseline (speedup 1.0000x reference)
"""MoE FFN block (pool -> LN -> top2 gate -> expert MLPs -> residual) on 8
Trainium2 NeuronCores.

Strategy: data-parallel over the batch (512 tokens/core). Dense expert
compute (all 8 experts for every token, weighted by the sparse combine
matrix), bf16 matmuls with fp32 accumulation; pooling, layernorm, gating,
combine weighting and the residual add are all fp32.
"""

import os
import sys

sys.path.insert(0, "/opt/trn_rl_repo")

import numpy as np
import ml_dtypes

import concourse.bass as bass
import concourse.bacc as bacc
import concourse.tile as tile
from concourse import mybir
from concourse.bass_utils import run_bass_kernel_spmd
from concourse.masks import make_identity

F32 = mybir.dt.float32
BF16 = mybir.dt.bfloat16

NCORES = 8
B = 4096
D = 1024
H = 4096
E = 8
HW = 64  # 8*8 spatial
EPS = 1e-5

TB = B // NCORES  # tokens per core
TT = TB // 128  # token tiles per core
DK = D // 128  # dim k-tiles
HM = H // 128  # hidden m-tiles
DRES = 64  # d-chunk size for streaming passes (64 d * 64 hw * 4B * 128p = 2MB)
NDC = D // DRES

_CACHE = {}


def _emit(nc):
    xin = nc.declare_dram_parameter("xs", [TB, D * HW], F32, isOutput=False)
    w1t = nc.declare_dram_parameter("w1t", [E, D, H], BF16, isOutput=False)
    w2t = nc.declare_dram_parameter("w2t", [E, H, D], BF16, isOutput=False)
    wgt = nc.declare_dram_parameter("wgt", [D, E], F32, isOutput=False)
    bg = nc.declare_dram_parameter("bg", [E, 1], F32, isOutput=False)
    b1 = nc.declare_dram_parameter("b1", [E, H], F32, isOutput=False)
    b2 = nc.declare_dram_parameter("b2", [E, D], F32, isOutput=False)
    gamma = nc.declare_dram_parameter("gamma", [1, D], F32, isOutput=False)
    beta = nc.declare_dram_parameter("beta", [1, D], F32, isOutput=False)
    out = nc.declare_dram_parameter("out", [TB, D * HW], F32, isOutput=True)
    comb_dram = nc.dram_tensor("comb_dram", [E, TB], F32)

    with tile.TileContext(nc) as tc:
        with (
            tc.tile_pool(name="const", bufs=1) as const,
            tc.tile_pool(name="resident", bufs=1) as resident,
            tc.tile_pool(name="psg", bufs=1, space="PSUM") as psg,
        ):
            # ---- constants ----
            ident = const.tile([128, 128], F32)
            make_identity(nc, ident)
            eps_t = const.tile([128, 1], F32)
            nc.vector.memset(eps_t[:], EPS)
            gamma_b = const.tile([128, D], F32)
            nc.gpsimd.dma_start(out=gamma_b[:], in_=gamma[:].to_broadcast((128, D)))
            beta_b = const.tile([128, D], F32)
            nc.gpsimd.dma_start(out=beta_b[:], in_=beta[:].to_broadcast((128, D)))
            # gate weights [D, E] -> [128, DK, E]
            wg_sb = const.tile([128, DK, E], F32)
            nc.sync.dma_start(
                out=wg_sb[:],
                in_=wgt[:].rearrange("(k p) e -> p k e", p=128),
            )
            bg_sb = const.tile([E, 1], F32)
            nc.sync.dma_start(out=bg_sb[:], in_=bg[:])
            b2_sb = const.tile([E, D], F32)
            nc.sync.dma_start(out=b2_sb[:], in_=b2[:])

            # live across phases B..E
            xnTb = [resident.tile([128, TB], BF16, tag=f"xnTb{k}", name=f"xnTb{k}") for k in range(DK)]
            combT = resident.tile([E, TB], F32, tag="combT")
            ffn = [resident.tile([128, TB], F32, tag=f"ffn{m}", name=f"ffn{m}") for m in range(DK)]

            with tc.tile_pool(name="phAB", bufs=1) as phAB:
                xnorm = [phAB.tile([128, D], F32, tag=f"xnorm{t}", name=f"xnorm{t}") for t in range(TT)]
                # ---- phase A: pool + layernorm ----
                with (
                    nc.named_scope("phaseA"),
                    tc.tile_pool(name="xstream", bufs=3) as xstream,
                    tc.tile_pool(name="stats", bufs=2) as stats,
                ):
                    for t in range(TT):
                        pool_t = xnorm[t]
                        ts = slice(t * 128, (t + 1) * 128)
                        for dc in range(NDC):
                            xt = xstream.tile([128, DRES, HW], F32, tag="xs")
                            nc.sync.dma_start(
                                out=xt[:],
                                in_=xin[ts, dc * DRES * HW : (dc + 1) * DRES * HW].rearrange(
                                    "p (d h) -> p d h", h=HW
                                ),
                            )
                            nc.vector.reduce_sum(
                                pool_t[:, dc * DRES : (dc + 1) * DRES],
                                xt[:],
                                mybir.AxisListType.X,
                            )
                        nc.scalar.mul(out=pool_t[:], in_=pool_t[:], mul=1.0 / HW)
                        st = stats.tile([128, 2, 6], F32, tag="st")
                        mv = stats.tile([128, 2], F32, tag="mv")
                        pg = pool_t[:].rearrange("p (s f) -> p s f", s=2)
                        for s in range(2):
                            nc.vector.bn_stats(out=st[:, s, :], in_=pg[:, s, :])
                        nc.vector.bn_aggr(out=mv[:], in_=st[:])
                        rstd = stats.tile([128, 1], F32, tag="rstd")
                        nc.scalar.activation(
                            out=rstd[:],
                            in_=mv[:, 1:2],
                            func=mybir.ActivationFunctionType.Sqrt,
                            bias=eps_t[:],
                            scale=1.0,
                        )
                        nc.vector.reciprocal(out=rstd[:], in_=rstd[:])
                        nc.vector.tensor_scalar(
                            out=pool_t[:],
                            in0=pool_t[:],
                            scalar1=mv[:, 0:1],
                            scalar2=rstd[:],
                            op0=mybir.AluOpType.subtract,
                            op1=mybir.AluOpType.mult,
                        )
                        nc.vector.tensor_mul(out=pool_t[:], in0=pool_t[:], in1=gamma_b[:])
                        nc.vector.tensor_add(out=pool_t[:], in0=pool_t[:], in1=beta_b[:])

                # ---- phase B: transpose + gate + top-2 combine ----
                with (
                    nc.named_scope("phaseB"),
                    tc.tile_pool(name="phB", bufs=1) as phB,
                    tc.tile_pool(name="gate", bufs=2) as gate,
                    tc.tile_pool(name="pst", bufs=2, space="PSUM") as pst,
                ):
                    xnT = [phB.tile([128, TB], F32, tag=f"xnT{k}", name=f"xnT{k}") for k in range(DK)]
                    for k in range(DK):
                        for t in range(TT):
                            pt = pst.tile([128, 128], F32, tag="ptr")
                            nc.tensor.transpose(
                                pt[:], xnorm[t][:, k * 128 : (k + 1) * 128], ident[:]
                            )
                            tsl = slice(t * 128, (t + 1) * 128)
                            nc.scalar.copy(out=xnT[k][:, tsl], in_=pt[:])
                            nc.vector.tensor_copy(out=xnTb[k][:, tsl], in_=pt[:])

                    logits_ps = psg.tile([E, TB], F32, tag="lps")
                    for k in range(DK):
                        nc.tensor.matmul(
                            logits_ps[:],
                            wg_sb[:, k, :],
                            xnT[k][:],
                            start=(k == 0),
                            stop=(k == DK - 1),
                        )
                    logitsT = gate.tile([E, TB], F32, tag="lT")
                    nc.vector.tensor_scalar(
                        out=logitsT[:],
                        in0=logits_ps[:],
                        scalar1=bg_sb[:],
                        scalar2=None,
                        op0=mybir.AluOpType.add,
                    )
                    for t in range(TT):
                        tsl = slice(t * 128, (t + 1) * 128)
                        lp = pst.tile([128, E], F32, tag="ptr")
                        nc.tensor.transpose(lp[:], logitsT[:, tsl], ident[:E, :E])
                        lg = gate.tile([128, E], F32, tag="lg")
                        nc.scalar.copy(out=lg[:], in_=lp[:])
                        mx = gate.tile([128, 8], F32, tag="mx")
                        nc.vector.max(out=mx[:], in_=lg[:])
                        d21 = gate.tile([128, 1], F32, tag="d21")
                        nc.vector.tensor_sub(out=d21[:], in0=mx[:, 1:2], in1=mx[:, 0:1])
                        s2 = gate.tile([128, 1], F32, tag="s2")
                        nc.scalar.activation(
                            out=s2[:], in_=d21[:], func=mybir.ActivationFunctionType.Sigmoid
                        )
                        s1 = gate.tile([128, 1], F32, tag="s1")
                        nc.scalar.activation(
                            out=s1[:],
                            in_=d21[:],
                            func=mybir.ActivationFunctionType.Sigmoid,
                            scale=-1.0,
                        )
                        m1b = gate.tile([128, E], F32, tag="m1b")
                        nc.vector.tensor_scalar(
                            out=m1b[:],
                            in0=lg[:],
                            scalar1=mx[:, 0:1],
                            scalar2=None,
                            op0=mybir.AluOpType.is_equal,
                        )
                        m2b = gate.tile([128, E], F32, tag="m2b")
                        nc.vector.tensor_scalar(
                            out=m2b[:],
                            in0=lg[:],
                            scalar1=mx[:, 1:2],
                            scalar2=None,
                            op0=mybir.AluOpType.is_equal,
                        )
                        comb = gate.tile([128, E], F32, tag="comb")
                        nc.vector.tensor_scalar_mul(out=m1b[:], in0=m1b[:], scalar1=s1[:])
                        nc.vector.tensor_scalar_mul(out=m2b[:], in0=m2b[:], scalar1=s2[:])
                        nc.vector.tensor_add(out=comb[:], in0=m1b[:], in1=m2b[:])
                        cp = pst.tile([E, 128], F32, tag="ptr")
                        nc.tensor.transpose(cp[:], comb[:], ident[:])
                        nc.scalar.copy(out=combT[:, tsl], in_=cp[:])
                    nc.sync.dma_start(out=comb_dram[:], in_=combT[:])

            # ---- phase C: dense expert FFN, accumulated in fp32 ----
            for m in range(DK):
                pb = psg.tile([128, TB], F32, tag="pb2")
                nc.tensor.matmul(
                    pb[:],
                    b2_sb[:, m * 128 : (m + 1) * 128],
                    combT[:],
                    start=True,
                    stop=True,
                )
                nc.scalar.copy(out=ffn[m][:], in_=pb[:])

            with (
                tc.tile_pool(name="phC", bufs=1) as phC,
                tc.tile_pool(name="w1s", bufs=2) as w1sp,
                tc.tile_pool(name="w2s", bufs=2) as w2sp,
                tc.tile_pool(name="fftmp", bufs=3) as fftmp,
                tc.tile_pool(name="psmm", bufs=2, space="PSUM") as psmm,
            ):
                hbf = [phC.tile([128, TB], BF16, tag=f"h{m}", name=f"h{m}") for m in range(HM)]
                HQ = H // 4  # hidden columns per layer-1 quarter
                DQ = D // 4  # dim columns per layer-2 quarter
                for e in range(E):
                    with nc.named_scope(f"exp{e}"):
                        cb = fftmp.tile([128, TB], F32, tag="cb")
                        nc.gpsimd.dma_start(
                            out=cb[:], in_=comb_dram[e : e + 1, :].to_broadcast((128, TB))
                        )
                        b1e = fftmp.tile([128, HM], F32, tag="b1e")
                        nc.sync.dma_start(
                            out=b1e[:],
                            in_=b1[e, :].rearrange("(m p) -> p m", p=128),
                        )

                        # layer 1: h = silu(x_norm @ w1[e].T + b1[e]); m-quarters,
                        # each quarter tile holds all 8 k-tiles for HQ h-columns
                        for q in range(4):
                            w1q = w1sp.tile([128, DK, HQ], BF16, tag="w1q", name=f"w1q{e}_{q}")
                            nc.sync.dma_start(
                                out=w1q[:],
                                in_=w1t[e, :, q * HQ : (q + 1) * HQ].rearrange(
                                    "(k p) h -> p k h", p=128
                                ),
                            )
                            for mi in range(HQ // 128):
                                m = q * (HQ // 128) + mi
                                ph = psmm.tile([128, TB], F32, tag="ph")
                                for k in range(DK):
                                    nc.tensor.matmul(
                                        ph[:],
                                        w1q[:, k, mi * 128 : (mi + 1) * 128],
                                        xnTb[k][:],
                                        start=(k == 0),
                                        stop=(k == DK - 1),
                                    )
                                nc.scalar.activation(
                                    out=hbf[m][:],
                                    in_=ph[:],
                                    func=mybir.ActivationFunctionType.Silu,
                                    bias=b1e[:, m : m + 1],
                                    scale=1.0,
                                )

                        # layer 2: y = h @ w2[e].T (combine-weighted, accumulated);
                        # m-quarters, each tile holds all 32 k-tiles for DQ d-columns
                        for q in range(4):
                            w2q = w2sp.tile([128, HM, DQ], BF16, tag="w2q", name=f"w2q{e}_{q}")
                            nc.sync.dma_start(
                                out=w2q[:],
                                in_=w2t[e, :, q * DQ : (q + 1) * DQ].rearrange(
                                    "(k p) d -> p k d", p=128
                                ),
                            )
                            for mi in range(DQ // 128):
                                m = q * (DQ // 128) + mi
                                py = psmm.tile([128, TB], F32, tag="py")
                                for k in range(HM):
                                    nc.tensor.matmul(
                                        py[:],
                                        w2q[:, k, mi * 128 : (mi + 1) * 128],
                                        hbf[k][:],
                                        start=(k == 0),
                                        stop=(k == HM - 1),
                                    )
                                yv = fftmp.tile([128, TB], F32, tag="yv")
                                nc.vector.tensor_mul(out=yv[:], in0=py[:], in1=cb[:])
                                nc.vector.tensor_add(out=ffn[m][:], in0=ffn[m][:], in1=yv[:])

            # ---- phase D: transpose ffn back to [token, d] ----
            with (
                nc.named_scope("phaseDE"),
                tc.tile_pool(name="phDE", bufs=1) as phDE,
                tc.tile_pool(name="pst2", bufs=2, space="PSUM") as pst2,
            ):
                ftok = [phDE.tile([128, D], F32, tag=f"ftok{t}", name=f"ftok{t}") for t in range(TT)]
                for m in range(DK):
                    for t in range(TT):
                        pt = pst2.tile([128, 128], F32, tag="ptr2")
                        nc.tensor.transpose(
                            pt[:], ffn[m][:, t * 128 : (t + 1) * 128], ident[:]
                        )
                        nc.scalar.copy(
                            out=ftok[t][:, m * 128 : (m + 1) * 128], in_=pt[:]
                        )

                # ---- phase E: residual broadcast-add, stream out ----
                with tc.tile_pool(name="xres", bufs=8) as xres:
                    for t in range(TT):
                        ts = slice(t * 128, (t + 1) * 128)
                        for dc in range(NDC):
                            xt = xres.tile([128, DRES, HW], F32, tag="xr")
                            nc.sync.dma_start(
                                out=xt[:],
                                in_=xin[ts, dc * DRES * HW : (dc + 1) * DRES * HW].rearrange(
                                    "p (d h) -> p d h", h=HW
                                ),
                            )
                            fsl = ftok[t][:, dc * DRES : (dc + 1) * DRES]
                            fb = bass.AP(
                                tensor=fsl.tensor,
                                offset=fsl.offset,
                                ap=[fsl.ap[0], fsl.ap[1], [0, HW]],
                            )
                            nc.vector.tensor_add(out=xt[:], in0=xt[:], in1=fb)
                            nc.scalar.dma_start(
                                out=out[ts, dc * DRES * HW : (dc + 1) * DRES * HW],
                                in_=xt[:].rearrange("p d h -> p (d h)"),
                            )
    nc.finalize()
    return nc


def _build():
    if "nc" not in _CACHE:
        nc = bacc.Bacc(None, target_bir_lowering=False, debug=False, num_devices=NCORES)
        _CACHE["nc"] = _emit(nc)
    return _CACHE["nc"]


def kernel(x, gamma, beta, wg, bg, w1, b1, w2, b2):
    nc = _build()

    x = np.asarray(x, dtype=np.float32)
    w1t = np.ascontiguousarray(
        np.asarray(w1).transpose(0, 2, 1).astype(ml_dtypes.bfloat16)
    )
    w2t = np.ascontiguousarray(
        np.asarray(w2).transpose(0, 2, 1).astype(ml_dtypes.bfloat16)
    )
    wgt = np.ascontiguousarray(np.asarray(wg, dtype=np.float32).T)
    bgr = np.asarray(bg, dtype=np.float32).reshape(E, 1)
    b1r = np.asarray(b1, dtype=np.float32)
    b2r = np.asarray(b2, dtype=np.float32)
    gam = np.asarray(gamma, dtype=np.float32).reshape(1, D)
    bet = np.asarray(beta, dtype=np.float32).reshape(1, D)

    xflat = x.reshape(B, D * HW)
    in_maps = []
    for c in range(NCORES):
        in_maps.append(
            {
                "xs": xflat[c * TB : (c + 1) * TB],
                "w1t": w1t,
                "w2t": w2t,
                "wgt": wgt,
                "bg": bgr,
                "b1": b1r,
                "b2": b2r,
                "gamma": gam,
                "beta": bet,
            }
        )

    trace = bool(int(os.environ.get("MOE_TRACE", "0")))
    res = run_bass_kernel_spmd(nc, in_maps, core_ids=list(range(NCORES)), trace=trace)
    _CACHE["last_result"] = res

    outp = np.empty((B, D, 8, 8), dtype=np.float32)
    for c in range(NCORES):
        outp[c * TB : (c + 1) * TB] = res.results[c]["out"].reshape(TB, D, 8, 8)
    return outp

